# revision 28
# baseline (speedup 1.0000x reference)
"""Trainium2 Bass kernel for GQA attention block (B=1, S=2048, DIM=4096,
32 q heads / 8 kv heads, head_dim 128, RoPE, causal, fused QKV + out proj).

Sharding: tensor-parallel over heads across 8 cores. Core i computes
q heads 4i..4i+3 and kv head i (one full GQA group), plus the wo
contribution of its 512 output columns; host sums the 8 partial outputs.

All matmul operands are bf16 (PSUM accumulation f32): same 1 cycle/row
PE rate as f32r but no <256-width rate cliff, half the DMA traffic, and
lower PE power (less GPIO throttling). End-to-end rel err ~3e-3 vs the
2e-2 gate.

Attention batches the core's 4 q heads into single 512-wide matmuls over
128-row s-tiles; exp covers a pair of t-blocks x 4 heads per activation
instruction. The output projection is fused into the attention loop as
act-independent PE filler after each s-tile, so the PE rides through exp
latency windows instead of stalling at s-tile drains. The causal triangle
is preloaded into PSUM by an identity matmul that opens the diagonal
block's accumulation group.
"""
import numpy as np
import ml_dtypes

import concourse.bass as bass
import concourse.mybir as mybir
import concourse.tile as tile
from concourse import bacc
from concourse.bass_utils import run_bass_kernel_spmd
from concourse.masks import make_identity

F32 = mybir.dt.float32
BF16 = mybir.dt.bfloat16
AF = mybir.ActivationFunctionType

B, S, DIM = 1, 2048, 4096
N_HEADS, N_KV_HEADS = 32, 8
HD = DIM // N_HEADS              # 128
N_CORES = 8
QH = N_HEADS // N_CORES          # 4 q heads per core
OC = QH * HD + 2 * HD            # 768 per-core qkv output columns
NS = S // 128                    # 16 s/t 128-blocks
ND = DIM // 128                  # 32 d-blocks
XSUB = 8                         # d-blocks per x sub-tile in phase 1
NXS = ND // XSUB                 # 4 x sub-tiles per s-block
WSUB = 2                         # d-blocks per w load chunk
NDC = DIM // 512                 # 8 output column chunks
SCALE = 1.0 / float(np.sqrt(HD))
MASK_NEG = -1.0e5


def _build_nc():
    nc = bacc.Bacc("TRN2", target_bir_lowering=False, debug=False)

    # host-pre-tiled inputs (see _prep_in_maps for layouts)
    xt = nc.dram_tensor("xt", [NS, NXS, 128, XSUB, 128], BF16,
                        kind="ExternalInput").ap()
    wt = nc.dram_tensor("wt", [128, ND, OC], BF16, kind="ExternalInput").ap()
    wot = nc.dram_tensor("wot", [128, NDC, QH, 512], BF16,
                         kind="ExternalInput").ap()
    cos5 = nc.dram_tensor("cos5", [128, NS, 5 * 64], F32,
                          kind="ExternalInput").ap()
    sin5 = nc.dram_tensor("sin5", [128, NS, 5 * 64], F32,
                          kind="ExternalInput").ap()
    cmask = nc.dram_tensor("cmask", [128, QH, 128], BF16,
                           kind="ExternalInput").ap()
    y = nc.dram_tensor("y", [S, DIM], BF16, kind="ExternalOutput").ap()

    with tile.TileContext(nc) as tc:
        _emit(tc, nc, xt, wt, wot, cos5, sin5, cmask, y)
    nc.compile()
    return nc


def _emit(tc, nc, xt, wt, wot, cos5, sin5, cmask, y):
    import contextlib

    with contextlib.ExitStack() as ctx:
        # ---------- long-lived tiles ----------
        keep = ctx.enter_context(tc.tile_pool(name="keep", bufs=1))
        # QT_all[:, h, :]: per-head roped Q transposed [d, s]; h=QH is roped K
        QT_all = keep.tile([128, QH + 1, S], BF16)
        V_all = keep.tile([128, NS, HD], BF16)          # V blocks [t, d]
        OT_all = keep.tile([128, QH, S], BF16)          # attn out transposed
        wo_sb = keep.tile([128, NDC, QH, 512], BF16)    # whole wo shard
        cos_all = keep.tile([128, NS, 320], F32)
        sin_all = keep.tile([128, NS, 320], F32)
        cmask4 = keep.tile([128, QH, 128], BF16)        # causal triangle x4
        ident = keep.tile([128, 128], BF16)
        make_identity(nc, ident)
        ones_f = keep.tile([128, 128], F32)
        nc.vector.memset(ones_f, 1.0)
        ones_b = keep.tile([128, 128], BF16)
        nc.vector.tensor_copy(ones_b, ones_f)
        # dummy exp: pull the act-table load (~1.3us) out of phase 2
        # (sin_all is overwritten by its DMA afterwards)
        nc.scalar.activation(sin_all[:, 0, 0:1], ones_f[:, 0:1], AF.Exp)


        # ---------- phase 1: qkv projection + RoPE + transposes ----------
        with (
            tc.tile_pool(name="p1w", bufs=1) as p1w,
            tc.tile_pool(name="p1x", bufs=2) as p1x,
            tc.tile_pool(name="p1t", bufs=1) as p1t,
            tc.tile_pool(name="p1ps", bufs=1, space="PSUM") as p1ps,
        ):
            # first x sub-tile before the w bulk so PE can start ASAP;
            # split so the first d-blocks land first
            x_first = p1x.tile([128, XSUB, 128], BF16, tag="x")
            nc.scalar.dma_start(x_first[:, 0:2], xt[0, 0, :, 0:2])
            nc.scalar.dma_start(x_first[:, 2:XSUB], xt[0, 0, :, 2:XSUB])
            w_sb = p1w.tile([128, ND, OC], BF16)
            # first two chunks are single-d-block so matmuls start earlier
            w_ranges = [(0, 1), (1, 2)] + [
                (c, c + WSUB) for c in range(2, ND, WSUB)]
            for c0, c1 in w_ranges:
                nc.sync.dma_start(w_sb[:, c0:c1, :], wt[:, c0:c1, :])
            # bulk tables after the w stream (needed later than w)
            nc.sync.dma_start(cos_all, cos5)
            nc.sync.dma_start(sin_all, sin5)
            nc.sync.dma_start(cmask4, cmask)
            nc.sync.dma_start(wo_sb, wot)

            def mm_pair(sb, db, x_sb, dbi):
                nc.tensor.matmul(
                    ps_qs[sb], lhsT=x_sb[:, dbi, :],
                    rhs=w_sb[:, db, 0:512],
                    start=(db == 0), stop=(db == ND - 1),
                )
                nc.tensor.matmul(
                    ps_kvs[sb], lhsT=x_sb[:, dbi, :],
                    rhs=w_sb[:, db, 512:768],
                    start=(db == 0), stop=(db == ND - 1),
                )

            GRP = 4
            groups = [list(range(g, min(g + GRP, NS))) for g in range(0, NS, GRP)]
            for group in groups:
                ps_qs = {}
                ps_kvs = {}
                x_tiles = {}
                for sb in group:
                    ps_qs[sb] = p1ps.tile([128, 512], F32, tag=f"psq{sb % GRP}",
                                          name=f"psq{sb}")
                    ps_kvs[sb] = p1ps.tile([128, 256], F32, tag=f"pskv{sb % GRP}",
                                           name=f"pskv{sb}")
                for xs in range(NXS - 1):
                    for sb in group:
                        if sb == 0 and xs == 0:
                            x_tiles[sb] = x_first
                        else:
                            x_tiles[sb] = p1x.tile(
                                [128, XSUB, 128], BF16, tag=f"x{sb % GRP}",
                                name=f"x{sb}_{xs}")
                            nc.scalar.dma_start(x_tiles[sb], xt[sb, xs])
                    for sb in group:
                        for dbi in range(XSUB):
                            mm_pair(sb, XSUB * xs + dbi, x_tiles[sb], dbi)
                # last x chunk s-block-major: each s-block's accumulation
                # finishes early so its RoPE (DVE) overlaps the next
                # s-block's closing matmuls instead of serializing at the
                # group boundary
                xs = NXS - 1
                for sb in group:
                    x_tiles[sb] = p1x.tile(
                        [128, XSUB, 128], BF16, tag=f"x{sb % GRP}",
                        name=f"x{sb}_{xs}")
                    nc.scalar.dma_start(x_tiles[sb], xt[sb, xs])
                qkr = {}
                prev = None
                for sb in group:
                    for dbi in range(XSUB):
                        mm_pair(sb, XSUB * xs + dbi, x_tiles[sb], dbi)
                    qkr[sb] = _rope(nc, p1t, cos_all, sin_all, sb,
                                    ps_qs[sb], ps_kvs[sb], V_all)
                    if prev is not None:
                        _transposes(nc, p1ps, prev, qkr.pop(prev),
                                    QT_all, ident)
                    prev = sb
                _transposes(nc, p1ps, prev, qkr.pop(prev), QT_all, ident)

        _emit_attn(tc, nc, ctx, QT_all, V_all, OT_all, ones_b, cmask4, ident,
                   wo_sb, y)


def _rope(nc, p1t, cos_all, sin_all, sb, ps_q, ps_kv, V_all):
    # RoPE (q: 4 heads = 512 cols; k: 128 cols), all on DVE
    cos_t = cos_all[:, sb, :]
    sin_t = sin_all[:, sb, :]

    qk_roped = p1t.tile([128, 640], BF16, tag=f"qkr{sb % 2}",
                        name=f"qkr{sb}")
    for part, ps_src, wid in (("q", ps_q, 512), ("k", ps_kv, 128)):
        nf = wid // 2
        off = 0 if part == "q" else 512
        pe = ps_src[:, 0:wid:2]
        po = ps_src[:, 1:wid:2]
        c = cos_t[:, 0:nf]
        sn = sin_t[:, 0:nf]
        t1 = p1t.tile([128, 256], F32, tag="t1")
        t2 = p1t.tile([128, 256], F32, tag="t2")
        nc.vector.tensor_mul(t1[:, 0:nf], pe, c)
        nc.vector.tensor_mul(t2[:, 0:nf], po, sn)
        nc.vector.tensor_sub(
            qk_roped[:, off + 0:off + wid:2], t1[:, 0:nf], t2[:, 0:nf])
        t3 = p1t.tile([128, 256], F32, tag="t3")
        t4 = p1t.tile([128, 256], F32, tag="t4")
        nc.vector.tensor_mul(t3[:, 0:nf], pe, sn)
        nc.vector.tensor_mul(t4[:, 0:nf], po, c)
        nc.vector.tensor_add(
            qk_roped[:, off + 1:off + wid:2], t3[:, 0:nf], t4[:, 0:nf])

    # V block: natural [t, d]; on scalar engine to keep DVE for RoPE
    nc.scalar.copy(V_all[:, sb, :], ps_kv[:, 128:256])
    return qk_roped


def _transposes(nc, p1ps, sb, qk_roped, QT_all, ident):
    # transpose roped q/k head-slices into QT_all (bf16: 1 cycle/row)
    for h in range(QH + 1):
        # borrow qkv accumulator slots (pool-tag reuse; tile's WAR
        # tracking orders this after the rope/V reads)
        tag = f"psq{sb % 4}" if h % 2 == 0 else f"pskv{sb % 4}"
        ps_t = p1ps.tile([128, 128], BF16, tag=tag, name=f"pst{sb}_{h}")
        nc.tensor.transpose(ps_t, qk_roped[:, 128 * h:128 * (h + 1)], ident)
        if h % 2 == 0:
            nc.scalar.copy(QT_all[:, h, 128 * sb:128 * (sb + 1)], ps_t)
        else:
            nc.vector.tensor_copy(QT_all[:, h, 128 * sb:128 * (sb + 1)], ps_t)


def _emit_attn(tc, nc, ctx, QT_all, V_all, OT_all, ones_b, cmask4, ident,
               wo_sb, y):
    # ---------- phase 2+3 fused: attention + output projection ----------
    # s-tiles are 128 rows; units are pairs of 128-row t-blocks sharing one
    # 2-bank score psum tile (one exp per pair). After each s-tile finishes,
    # the output-projection matmuls for an already-normalized s-block are
    # emitted as act-independent PE filler, so the PE rides through every
    # exp-latency window instead of stalling at s-tile drains.
    # PSUM: sc x2 (4 banks) + av + den (2) + psy x2 (2) = 8.
    with (
        tc.tile_pool(name="p2et", bufs=1) as p2et,
        tc.tile_pool(name="p2t", bufs=2) as p2t,
        tc.tile_pool(name="p3y", bufs=6) as p3y,
        tc.tile_pool(name="p2sc", bufs=1, space="PSUM") as p2sc,
        tc.tile_pool(name="p2acc", bufs=1, space="PSUM") as p2acc,
        tc.tile_pool(name="p2y", bufs=1, space="PSUM") as p2y,
    ):
        # global pair-unit queue across all s-tiles
        units = []                    # (st, j0, nj_in_pair)
        for st in range(NS):
            nj = st + 1
            for j0 in range(0, nj, 2):
                units.append((st, j0, min(2, nj - j0)))

        state = {}                    # st -> (ET, avden, sc tiles by unit)
        sc_tiles = {}
        tagc = [0]

        def emit_scores(u):
            st, j0, np_ = units[u]
            if st not in state:
                ET = p2et.tile([128, NS, QH, 128], BF16, tag="et",
                               name=f"et{st}")
                av = p2acc.tile([128, QH, 128], F32, tag="av",
                                name=f"av{st}")
                den = p2acc.tile([128, QH, 128], F32, tag="den",
                                 name=f"den{st}")
                state[st] = (ET, av, den)
            ps = p2sc.tile([128, 2, QH, 128], F32, tag=f"sc{tagc[0] % 2}",
                           name=f"sc{st}_{j0}")
            tagc[0] += 1
            sc_tiles[u] = ps
            for ji in range(np_):
                j = j0 + ji
                diag = j == st
                if diag:
                    # causal triangle preloaded by the PE itself (identity
                    # matmul opens the accumulation group), so the exp
                    # never waits on a cross-engine mask add
                    nc.tensor.matmul(
                        ps[:, ji], lhsT=ident, rhs=cmask4,
                        start=True, stop=False, skip_group_check=True,
                    )
                nc.tensor.matmul(
                    ps[:, ji],
                    lhsT=QT_all[:, QH, 128 * j:128 * (j + 1)],
                    rhs=QT_all[:, 0:QH, 128 * st:128 * (st + 1)],
                    start=not diag, stop=True, skip_group_check=True,
                )

        def emit_tail(u):
            st, j0, np_ = units[u]
            ET, av, den = state[st]
            ps = sc_tiles.pop(u)
            nj = st + 1
            nc.scalar.activation(
                ET[:, j0:j0 + np_], ps[:, 0:np_], AF.Exp, scale=SCALE)
            for ji in range(np_):
                j = j0 + ji
                nc.tensor.matmul(
                    av, lhsT=V_all[:, j, :], rhs=ET[:, j],
                    start=(j == 0), stop=(j == nj - 1),
                    skip_group_check=True,
                )
                nc.tensor.matmul(
                    den, lhsT=ones_b, rhs=ET[:, j],
                    start=(j == 0), stop=(j == nj - 1),
                    skip_group_check=True,
                )
            if j0 + np_ == nj:        # s-tile done: normalize
                den_r = p2t.tile([128, QH, 128], F32, tag="denr")
                nc.vector.reciprocal_approx_fast(den_r, den)
                nc.vector.tensor_mul(
                    OT_all[:, :, 128 * st:128 * (st + 1)], av, den_r)
                del state[st]
                return st
            return None

        def emit_p3(sb):
            # output projection for one (long-normalized) s-block
            for dc in range(NDC):
                ps_y = p2y.tile([128, 512], F32, tag=f"y{dc % 2}",
                                name=f"psy{sb}_{dc}")
                for ob in range(QH):
                    nc.tensor.matmul(
                        ps_y,
                        lhsT=OT_all[:, ob, 128 * sb:128 * (sb + 1)],
                        rhs=wo_sb[:, dc, ob, :],
                        start=(ob == 0), stop=(ob == QH - 1),
                    )
                y_sb = p3y.tile([128, 512], BF16, tag="ysb")
                if dc % 2 == 0:
                    nc.vector.tensor_copy(y_sb, ps_y)
                else:
                    nc.scalar.copy(y_sb, ps_y)
                # last blocks on the idle HWDGE queues to shorten the tail
                if sb >= NS - 2:
                    eng = nc.sync if dc % 2 == 0 else nc.scalar
                else:
                    eng = nc.gpsimd
                eng.dma_start(
                    y[128 * sb:128 * (sb + 1), 512 * dc:512 * (dc + 1)], y_sb)

        LOOK = 2
        for u in range(len(units)):
            emit_scores(u)
            if u >= LOOK:
                done = emit_tail(u - LOOK)
                if done is not None and done >= 1:
                    emit_p3(done - 1)
        # drain: last two units (both in the final s-tile); p3 blocks
        # interleave so the closing exp latencies stay covered
        emit_tail(len(units) - 2)
        emit_p3(NS - 2)
        emit_tail(len(units) - 1)
        emit_p3(NS - 1)


_NC_CACHE = None


def _get_nc():
    global _NC_CACHE
    if _NC_CACHE is None:
        _NC_CACHE = _build_nc()
    return _NC_CACHE


def _prep_in_maps(x, freqs_cos, freqs_sin, wqkv, wo):
    bf = ml_dtypes.bfloat16
    xT = x.reshape(S, DIM).T.astype(bf)                        # [DIM, S]
    # xt[sb, xs, p, n, s] = xT[128*(XSUB*xs+n)+p, 128*sb+s]
    xt = np.ascontiguousarray(
        xT.reshape(NXS, XSUB, 128, NS, 128).transpose(3, 0, 2, 1, 4))
    # cos5[p, sb, f] = tile5(freqs_cos)[128*sb + p, f]
    cos5 = np.ascontiguousarray(
        np.tile(freqs_cos, (1, 5)).reshape(NS, 128, 320).transpose(1, 0, 2))
    sin5 = np.ascontiguousarray(
        np.tile(freqs_sin, (1, 5)).reshape(NS, 128, 320).transpose(1, 0, 2))

    # causal triangle for a diagonal 128-block, replicated over 4 heads
    tl = np.arange(128)[:, None]
    sl = np.arange(128)[None, :]
    tri = np.where(sl >= tl, 0.0, MASK_NEG).astype(np.float32)
    cm = np.ascontiguousarray(
        np.broadcast_to(tri[:, None, :], (128, QH, 128)).astype(bf))

    in_maps = []
    for i in range(N_CORES):
        wq = wqkv[QH * HD * i: QH * HD * (i + 1)]               # [512, DIM]
        wk = wqkv[N_HEADS * HD + HD * i: N_HEADS * HD + HD * (i + 1)]
        wv = wqkv[N_HEADS * HD + N_KV_HEADS * HD + HD * i:
                  N_HEADS * HD + N_KV_HEADS * HD + HD * (i + 1)]
        wT = np.concatenate([wq, wk, wv], axis=0).T.astype(bf)  # [DIM, 768]
        # wt[p, db, o] = wT[128*db+p, o]
        wt = np.ascontiguousarray(wT.reshape(ND, 128, OC).transpose(1, 0, 2))
        woT = wo[:, QH * HD * i: QH * HD * (i + 1)].T.astype(bf)  # [512, DIM]
        # wot[p, dc, ob, j] = woT[128*ob+p, 512*dc+j]
        wot = np.ascontiguousarray(
            woT.reshape(QH, 128, NDC, 512).transpose(1, 2, 0, 3))
        in_maps.append({
            "xt": xt, "wt": wt, "wot": wot,
            "cos5": cos5, "sin5": sin5, "cmask": cm,
        })
    return in_maps


def kernel(x, freqs_cos, freqs_sin, mask, wqkv, wo, _want_trace=False):
    x = np.asarray(x, np.float32)
    freqs_cos = np.asarray(freqs_cos, np.float32)
    freqs_sin = np.asarray(freqs_sin, np.float32)
    wqkv = np.asarray(wqkv, np.float32)
    wo = np.asarray(wo, np.float32)

    nc = _get_nc()
    in_maps = _prep_in_maps(x, freqs_cos, freqs_sin, wqkv, wo)
    res = run_bass_kernel_spmd(
        nc, in_maps, core_ids=list(range(N_CORES)), trace=_want_trace,
    )
    out = np.zeros((S, DIM), np.float32)
    for r in res.results:
        out += np.asarray(r["y"]).astype(np.float32)
    if _want_trace:
        kernel._last_results = res
    return out.reshape(B, S, DIM)


# revision 29
# speedup vs baseline: 1.1989x; 1.1989x over previous
"""Trainium2 Bass kernel for GQA attention block (B=1, S=2048, DIM=4096,
32 q heads / 8 kv heads, head_dim 128, RoPE, causal, fused QKV + out proj).

Sharding: tensor-parallel over heads across 8 cores. Core i computes
q heads 4i..4i+3 and kv head i (one full GQA group), plus the wo
contribution of its 512 output columns; host sums the 8 partial outputs.

All matmul operands are bf16 (PSUM accumulation f32): same 1 cycle/row
PE rate as f32r but no <256-width rate cliff, half the DMA traffic, and
lower PE power (less GPIO throttling). End-to-end rel err ~3e-3 vs the
2e-2 gate.

Attention batches the core's 4 q heads into single 512-wide matmuls over
128-row s-tiles; exp covers a pair of t-blocks x 4 heads per activation
instruction. The output projection is fused into the attention loop as
act-independent PE filler after each s-tile, so the PE rides through exp
latency windows instead of stalling at s-tile drains. The causal triangle
is preloaded into PSUM by an identity matmul that opens the diagonal
block's accumulation group.
"""
import numpy as np
import ml_dtypes

import concourse.bass as bass
import concourse.mybir as mybir
import concourse.tile as tile
from concourse import bacc
from concourse.bass_utils import run_bass_kernel_spmd
from concourse.masks import make_identity

F32 = mybir.dt.float32
BF16 = mybir.dt.bfloat16
AF = mybir.ActivationFunctionType

B, S, DIM = 1, 2048, 4096
N_HEADS, N_KV_HEADS = 32, 8
HD = DIM // N_HEADS              # 128
N_CORES = 8
QH = N_HEADS // N_CORES          # 4 q heads per core
OC = QH * HD + 2 * HD            # 768 per-core qkv output columns
NS = S // 128                    # 16 s/t 128-blocks
ND = DIM // 128                  # 32 d-blocks
XSUB = 8                         # d-blocks per x sub-tile in phase 1
NXS = ND // XSUB                 # 4 x sub-tiles per s-block
WSUB = 2                         # d-blocks per w load chunk
NDC = DIM // 512                 # 8 output column chunks
SCALE = 1.0 / float(np.sqrt(HD))
MASK_NEG = -1.0e5


def _build_nc():
    nc = bacc.Bacc("TRN2", target_bir_lowering=False, debug=False)

    # host-pre-tiled inputs (see _prep_in_maps for layouts)
    xt = nc.dram_tensor("xt", [NS, NXS, 128, XSUB, 128], BF16,
                        kind="ExternalInput").ap()
    wt = nc.dram_tensor("wt", [128, ND, OC], BF16, kind="ExternalInput").ap()
    wot = nc.dram_tensor("wot", [128, NDC, QH, 512], BF16,
                         kind="ExternalInput").ap()
    cos5 = nc.dram_tensor("cos5", [128, NS, 5 * 64], F32,
                          kind="ExternalInput").ap()
    sin5 = nc.dram_tensor("sin5", [128, NS, 5 * 64], F32,
                          kind="ExternalInput").ap()
    cmask = nc.dram_tensor("cmask", [128, QH, 128], BF16,
                           kind="ExternalInput").ap()
    y = nc.dram_tensor("y", [S, DIM], BF16, kind="ExternalOutput").ap()

    with tile.TileContext(nc) as tc:
        _emit(tc, nc, xt, wt, wot, cos5, sin5, cmask, y)
    nc.compile()
    return nc


def _emit(tc, nc, xt, wt, wot, cos5, sin5, cmask, y):
    import contextlib

    with contextlib.ExitStack() as ctx:
        # ---------- long-lived tiles ----------
        keep = ctx.enter_context(tc.tile_pool(name="keep", bufs=1))
        # QT_all[:, h, :]: per-head roped Q transposed [d, s]; h=QH is roped K
        QT_all = keep.tile([128, QH + 1, S], BF16)
        V_all = keep.tile([128, NS, HD], BF16)          # V blocks [t, d]
        OT_all = keep.tile([128, QH, S], BF16)          # attn out transposed
        wo_sb = keep.tile([128, NDC, QH, 512], BF16)    # whole wo shard
        cos_all = keep.tile([128, NS, 320], F32)
        sin_all = keep.tile([128, NS, 320], F32)
        cmask4 = keep.tile([128, QH, 128], BF16)        # causal triangle x4
        ident = keep.tile([128, 128], BF16)
        make_identity(nc, ident)
        ones_f = keep.tile([128, 128], F32)
        nc.vector.memset(ones_f, 1.0)
        ones_b = keep.tile([128, 128], BF16)
        nc.vector.tensor_copy(ones_b, ones_f)
        # dummy exp: pull the act-table load (~1.3us) out of phase 2
        # (sin_all is overwritten by its DMA afterwards)
        nc.scalar.activation(sin_all[:, 0, 0:1], ones_f[:, 0:1], AF.Exp)


        # ---------- phase 1: qkv projection + RoPE + transposes ----------
        with (
            tc.tile_pool(name="p1w", bufs=1) as p1w,
            tc.tile_pool(name="p1x", bufs=2) as p1x,
            tc.tile_pool(name="p1t", bufs=1) as p1t,
            tc.tile_pool(name="p1ps", bufs=1, space="PSUM") as p1ps,
        ):
            # first x sub-tile before the w bulk so PE can start ASAP;
            # split so the first d-blocks land first
            x_first = p1x.tile([128, XSUB, 128], BF16, tag="x")
            nc.scalar.dma_start(x_first[:, 0:2], xt[0, 0, :, 0:2])
            nc.scalar.dma_start(x_first[:, 2:XSUB], xt[0, 0, :, 2:XSUB])
            w_sb = p1w.tile([128, ND, OC], BF16)
            # first two chunks are single-d-block so matmuls start earlier
            w_ranges = [(0, 1), (1, 2)] + [
                (c, c + WSUB) for c in range(2, ND, WSUB)]
            for c0, c1 in w_ranges:
                nc.sync.dma_start(w_sb[:, c0:c1, :], wt[:, c0:c1, :])
            # bulk tables after the w stream (needed later than w)
            nc.sync.dma_start(cos_all, cos5)
            nc.sync.dma_start(sin_all, sin5)
            nc.sync.dma_start(cmask4, cmask)
            nc.sync.dma_start(wo_sb, wot)

            def mm_pair(sb, db, x_sb, dbi):
                nc.tensor.matmul(
                    ps_qs[sb], lhsT=x_sb[:, dbi, :],
                    rhs=w_sb[:, db, 0:512],
                    start=(db == 0), stop=(db == ND - 1),
                )
                nc.tensor.matmul(
                    ps_kvs[sb], lhsT=x_sb[:, dbi, :],
                    rhs=w_sb[:, db, 512:768],
                    start=(db == 0), stop=(db == ND - 1),
                )

            GRP = 4
            groups = [list(range(g, min(g + GRP, NS))) for g in range(0, NS, GRP)]
            for group in groups:
                ps_qs = {}
                ps_kvs = {}
                x_tiles = {}
                for sb in group:
                    ps_qs[sb] = p1ps.tile([128, 512], F32, tag=f"psq{sb % GRP}",
                                          name=f"psq{sb}")
                    ps_kvs[sb] = p1ps.tile([128, 256], F32, tag=f"pskv{sb % GRP}",
                                           name=f"pskv{sb}")
                for xs in range(NXS - 1):
                    for sb in group:
                        if sb == 0 and xs == 0:
                            x_tiles[sb] = x_first
                        else:
                            x_tiles[sb] = p1x.tile(
                                [128, XSUB, 128], BF16, tag=f"x{sb % GRP}",
                                name=f"x{sb}_{xs}")
                            nc.scalar.dma_start(x_tiles[sb], xt[sb, xs])
                    for sb in group:
                        for dbi in range(XSUB):
                            mm_pair(sb, XSUB * xs + dbi, x_tiles[sb], dbi)
                # last x chunk s-block-major: each s-block's accumulation
                # finishes early so its RoPE (DVE) overlaps the next
                # s-block's closing matmuls instead of serializing at the
                # group boundary
                xs = NXS - 1
                for sb in group:
                    x_tiles[sb] = p1x.tile(
                        [128, XSUB, 128], BF16, tag=f"x{sb % GRP}",
                        name=f"x{sb}_{xs}")
                    nc.scalar.dma_start(x_tiles[sb], xt[sb, xs])
                qkr = {}
                prev = None
                for sb in group:
                    for dbi in range(XSUB):
                        mm_pair(sb, XSUB * xs + dbi, x_tiles[sb], dbi)
                    qkr[sb] = _rope(nc, p1t, cos_all, sin_all, sb,
                                    ps_qs[sb], ps_kvs[sb], V_all)
                    if prev is not None:
                        _transposes(nc, p1ps, prev, qkr.pop(prev),
                                    QT_all, ident)
                    prev = sb
                _transposes(nc, p1ps, prev, qkr.pop(prev), QT_all, ident)

        _emit_attn(tc, nc, ctx, QT_all, V_all, OT_all, ones_b, cmask4, ident,
                   wo_sb, y)


def _rope(nc, p1t, cos_all, sin_all, sb, ps_q, ps_kv, V_all):
    # RoPE (q: 4 heads = 512 cols; k: 128 cols), all on DVE
    cos_t = cos_all[:, sb, :]
    sin_t = sin_all[:, sb, :]

    qk_roped = p1t.tile([128, 640], BF16, tag=f"qkr{sb % 2}",
                        name=f"qkr{sb}")
    for part, ps_src, wid in (("q", ps_q, 512), ("k", ps_kv, 128)):
        nf = wid // 2
        off = 0 if part == "q" else 512
        pe = ps_src[:, 0:wid:2]
        po = ps_src[:, 1:wid:2]
        c = cos_t[:, 0:nf]
        sn = sin_t[:, 0:nf]
        t1 = p1t.tile([128, 256], F32, tag="t1")
        t2 = p1t.tile([128, 256], F32, tag="t2")
        nc.vector.tensor_mul(t1[:, 0:nf], pe, c)
        nc.vector.tensor_mul(t2[:, 0:nf], po, sn)
        nc.vector.tensor_sub(
            qk_roped[:, off + 0:off + wid:2], t1[:, 0:nf], t2[:, 0:nf])
        t3 = p1t.tile([128, 256], F32, tag="t3")
        t4 = p1t.tile([128, 256], F32, tag="t4")
        nc.vector.tensor_mul(t3[:, 0:nf], pe, sn)
        nc.vector.tensor_mul(t4[:, 0:nf], po, c)
        nc.vector.tensor_add(
            qk_roped[:, off + 1:off + wid:2], t3[:, 0:nf], t4[:, 0:nf])

    # V block: natural [t, d]; on scalar engine to keep DVE for RoPE
    nc.scalar.copy(V_all[:, sb, :], ps_kv[:, 128:256])
    return qk_roped


def _transposes(nc, p1ps, sb, qk_roped, QT_all, ident):
    # transpose roped q/k head-slices into QT_all (bf16: 1 cycle/row)
    for h in range(QH + 1):
        # borrow qkv accumulator slots (pool-tag reuse; tile's WAR
        # tracking orders this after the rope/V reads)
        tag = f"psq{sb % 4}" if h % 2 == 0 else f"pskv{sb % 4}"
        ps_t = p1ps.tile([128, 128], BF16, tag=tag, name=f"pst{sb}_{h}")
        nc.tensor.transpose(ps_t, qk_roped[:, 128 * h:128 * (h + 1)], ident)
        if h % 2 == 0:
            nc.scalar.copy(QT_all[:, h, 128 * sb:128 * (sb + 1)], ps_t)
        else:
            nc.vector.tensor_copy(QT_all[:, h, 128 * sb:128 * (sb + 1)], ps_t)


def _emit_attn(tc, nc, ctx, QT_all, V_all, OT_all, ones_b, cmask4, ident,
               wo_sb, y):
    # ---------- phase 2+3 fused: attention + output projection ----------
    # s-tiles are 128 rows; units are pairs of 128-row t-blocks sharing one
    # 2-bank score psum tile (one exp per pair). After each s-tile finishes,
    # the output-projection matmuls for an already-normalized s-block are
    # emitted as act-independent PE filler, so the PE rides through every
    # exp-latency window instead of stalling at s-tile drains.
    # PSUM: sc x2 (4 banks) + av + den (2) + psy x2 (2) = 8.
    with (
        tc.tile_pool(name="p2et", bufs=1) as p2et,
        tc.tile_pool(name="p2t", bufs=2) as p2t,
        tc.tile_pool(name="p3y", bufs=6) as p3y,
        tc.tile_pool(name="p2sc", bufs=1, space="PSUM") as p2sc,
        tc.tile_pool(name="p2acc", bufs=1, space="PSUM") as p2acc,
        tc.tile_pool(name="p2y", bufs=1, space="PSUM") as p2y,
    ):
        # global pair-unit queue across all s-tiles
        units = []                    # (st, j0, nj_in_pair)
        for st in range(NS):
            nj = st + 1
            for j0 in range(0, nj, 2):
                units.append((st, j0, min(2, nj - j0)))

        state = {}                    # st -> (ET, avden, sc tiles by unit)
        sc_tiles = {}
        tagc = [0]

        def emit_scores(u):
            st, j0, np_ = units[u]
            if st not in state:
                ET = p2et.tile([128, NS, QH, 128], BF16, tag="et",
                               name=f"et{st}")
                av = p2acc.tile([128, QH, 128], F32, tag="av",
                                name=f"av{st}")
                den = p2acc.tile([128, QH, 128], F32, tag="den",
                                 name=f"den{st}")
                state[st] = (ET, av, den)
            ps = p2sc.tile([128, 2, QH, 128], F32, tag=f"sc{tagc[0] % 2}",
                           name=f"sc{st}_{j0}")
            tagc[0] += 1
            sc_tiles[u] = ps
            for ji in range(np_):
                j = j0 + ji
                diag = j == st
                if diag:
                    # causal triangle preloaded by the PE itself (identity
                    # matmul opens the accumulation group), so the exp
                    # never waits on a cross-engine mask add
                    nc.tensor.matmul(
                        ps[:, ji], lhsT=ident, rhs=cmask4,
                        start=True, stop=False, skip_group_check=True,
                    )
                nc.tensor.matmul(
                    ps[:, ji],
                    lhsT=QT_all[:, QH, 128 * j:128 * (j + 1)],
                    rhs=QT_all[:, 0:QH, 128 * st:128 * (st + 1)],
                    start=not diag, stop=True, skip_group_check=True,
                )

        def emit_tail(u):
            st, j0, np_ = units[u]
            ET, av, den = state[st]
            ps = sc_tiles.pop(u)
            nj = st + 1
            nc.scalar.activation(
                ET[:, j0:j0 + np_], ps[:, 0:np_], AF.Exp, scale=SCALE)
            for ji in range(np_):
                j = j0 + ji
                nc.tensor.matmul(
                    av, lhsT=V_all[:, j, :], rhs=ET[:, j],
                    start=(j == 0), stop=(j == nj - 1),
                    skip_group_check=True,
                )
                nc.tensor.matmul(
                    den, lhsT=ones_b, rhs=ET[:, j],
                    start=(j == 0), stop=(j == nj - 1),
                    skip_group_check=True,
                )
            if j0 + np_ == nj:        # s-tile done: normalize
                den_r = p2t.tile([128, QH, 128], F32, tag="denr")
                nc.vector.reciprocal_approx_fast(den_r, den)
                nc.vector.tensor_mul(
                    OT_all[:, :, 128 * st:128 * (st + 1)], av, den_r)
                del state[st]
                return st
            return None

        def emit_p3(sb, dcs=None):
            # output projection for one (long-normalized) s-block
            for dc in (range(NDC) if dcs is None else dcs):
                ps_y = p2y.tile([128, 512], F32, tag=f"y{dc % 2}",
                                name=f"psy{sb}_{dc}")
                for ob in range(QH):
                    nc.tensor.matmul(
                        ps_y,
                        lhsT=OT_all[:, ob, 128 * sb:128 * (sb + 1)],
                        rhs=wo_sb[:, dc, ob, :],
                        start=(ob == 0), stop=(ob == QH - 1),
                    )
                y_sb = p3y.tile([128, 512], BF16, tag="ysb")
                if dc % 2 == 0:
                    nc.vector.tensor_copy(y_sb, ps_y)
                else:
                    nc.scalar.copy(y_sb, ps_y)
                # last blocks on the idle HWDGE queues to shorten the tail
                if sb >= NS - 2:
                    eng = nc.sync if dc % 2 == 0 else nc.scalar
                else:
                    eng = nc.gpsimd
                eng.dma_start(
                    y[128 * sb:128 * (sb + 1), 512 * dc:512 * (dc + 1)], y_sb)

        LOOK = 2
        for u in range(len(units)):
            emit_scores(u)
            if u >= LOOK:
                done = emit_tail(u - LOOK)
                if done is not None and done >= 1:
                    emit_p3(done - 1)
        # drain: last two units (both in the final s-tile); halves of the
        # sb=14 projection block lead each tail so the closing exp
        # latencies (and the final AV/den waits) stay covered by filler
        emit_p3(NS - 2, range(0, NDC // 2))
        emit_tail(len(units) - 2)
        emit_p3(NS - 2, range(NDC // 2, NDC))
        emit_tail(len(units) - 1)
        emit_p3(NS - 1)


_NC_CACHE = None


def _get_nc():
    global _NC_CACHE
    if _NC_CACHE is None:
        _NC_CACHE = _build_nc()
    return _NC_CACHE


def _prep_in_maps(x, freqs_cos, freqs_sin, wqkv, wo):
    bf = ml_dtypes.bfloat16
    xT = x.reshape(S, DIM).T.astype(bf)                        # [DIM, S]
    # xt[sb, xs, p, n, s] = xT[128*(XSUB*xs+n)+p, 128*sb+s]
    xt = np.ascontiguousarray(
        xT.reshape(NXS, XSUB, 128, NS, 128).transpose(3, 0, 2, 1, 4))
    # cos5[p, sb, f] = tile5(freqs_cos)[128*sb + p, f]
    cos5 = np.ascontiguousarray(
        np.tile(freqs_cos, (1, 5)).reshape(NS, 128, 320).transpose(1, 0, 2))
    sin5 = np.ascontiguousarray(
        np.tile(freqs_sin, (1, 5)).reshape(NS, 128, 320).transpose(1, 0, 2))

    # causal triangle for a diagonal 128-block, replicated over 4 heads
    tl = np.arange(128)[:, None]
    sl = np.arange(128)[None, :]
    tri = np.where(sl >= tl, 0.0, MASK_NEG).astype(np.float32)
    cm = np.ascontiguousarray(
        np.broadcast_to(tri[:, None, :], (128, QH, 128)).astype(bf))

    in_maps = []
    for i in range(N_CORES):
        wq = wqkv[QH * HD * i: QH * HD * (i + 1)]               # [512, DIM]
        wk = wqkv[N_HEADS * HD + HD * i: N_HEADS * HD + HD * (i + 1)]
        wv = wqkv[N_HEADS * HD + N_KV_HEADS * HD + HD * i:
                  N_HEADS * HD + N_KV_HEADS * HD + HD * (i + 1)]
        wT = np.concatenate([wq, wk, wv], axis=0).T.astype(bf)  # [DIM, 768]
        # wt[p, db, o] = wT[128*db+p, o]
        wt = np.ascontiguousarray(wT.reshape(ND, 128, OC).transpose(1, 0, 2))
        woT = wo[:, QH * HD * i: QH * HD * (i + 1)].T.astype(bf)  # [512, DIM]
        # wot[p, dc, ob, j] = woT[128*ob+p, 512*dc+j]
        wot = np.ascontiguousarray(
            woT.reshape(QH, 128, NDC, 512).transpose(1, 2, 0, 3))
        in_maps.append({
            "xt": xt, "wt": wt, "wot": wot,
            "cos5": cos5, "sin5": sin5, "cmask": cm,
        })
    return in_maps


def kernel(x, freqs_cos, freqs_sin, mask, wqkv, wo, _want_trace=False):
    x = np.asarray(x, np.float32)
    freqs_cos = np.asarray(freqs_cos, np.float32)
    freqs_sin = np.asarray(freqs_sin, np.float32)
    wqkv = np.asarray(wqkv, np.float32)
    wo = np.asarray(wo, np.float32)

    nc = _get_nc()
    in_maps = _prep_in_maps(x, freqs_cos, freqs_sin, wqkv, wo)
    res = run_bass_kernel_spmd(
        nc, in_maps, core_ids=list(range(N_CORES)), trace=_want_trace,
    )
    out = np.zeros((S, DIM), np.float32)
    for r in res.results:
        out += np.asarray(r["y"]).astype(np.float32)
    if _want_trace:
        kernel._last_results = res
    return out.reshape(B, S, DIM)
